# revision 5
# baseline (speedup 1.0000x reference)
"""Trainium2 Bass kernel for nn_NormalizedDistanceLoss.

Math: for x in R^{N x D}, with sq_i = ||x_i||^2, the strict-upper-triangle
sum of pairwise squared distances collapses algebraically:

    sum_{i<j} (sq_i + sq_j - 2 x_i.x_j) = N * S - ||s||^2

where S = sum_i sq_i and s = sum_i x_i (column sums).  So the loss

    loss = sum_masked_dist / (sqrt(max_i sq_i) * N(N-1)/2)

needs only one pass over x: per-row squared norms (for S and the max)
and column sums (for s).  Each of the 8 cores reduces its 1024-row block;
the host combines tiny per-core partials (a few KB per core).

Per-core device kernel (block = 1024 x 512 f32), raw bass (no TileContext,
manual semaphores) to minimize fixed framework overhead:
  - SBUF X[128, 8, 512]: partition p holds DRAM rows p*8..p*8+7 (16KB
    contiguous per partition).
  - Input DMA: each HWDGE queue generates descriptors at only ~27ns each,
    so chunk layout balances descriptor counts (128 per chunk): tiles 4-5
    on the SP ring set (4KB descriptors, lands first), then tiles 0-3
    (8KB descriptors) and tiles 6-7 (4KB) both on the Act ring set.  The
    last chunk carries only one pair-add and two squares of tail work.
  - Row squared norms: fused square+row-accumulate; ACT (Square
    activation + accum_out) takes 5 tiles, DVE (scalar_tensor_tensor +
    accum_out) takes 3 plus the pair adds, with the last pair's add
    hoisted ahead of the remaining DVE square so the final matmul ->
    PSUM-copy -> colsum-DMA chain starts as early as possible.
  - Column sums: DVE adds tile pairs into bf16 tiles; the otherwise-idle
    PE contracts the 128 partitions with a ones-vector matmul accumulated
    in one PSUM bank; ACT copies PSUM->SBUF.  bf16 pair rounding perturbs
    the loss ~1e-8 relative.
  - The output DMAs are issued but NOT waited on: the NEFF epilogue
    drains the DMA queues, so the output flight overlaps the fixed
    semaphore-clear teardown (~6us of EVENT_SEMAPHORE clears that
    dominates the non-body time).
"""

import sys

if "/opt/trn_rl_repo" not in sys.path:
    sys.path.insert(0, "/opt/trn_rl_repo")

import numpy as np

from concourse import bacc, mybir

N = 8192
D = 512
NCORES = 8
ROWS = N // NCORES  # 1024 rows per core
P = 128
T = ROWS // P  # 8 row-tiles of 512

_nc_cache = []


def _build_nc():
    f32 = mybir.dt.float32
    bf16 = mybir.dt.bfloat16
    nc = bacc.Bacc(
        "TRN2",
        target_bir_lowering=False,
        debug=False,
        num_devices=NCORES,
    )
    x_dram = nc.dram_tensor("x_blk", [ROWS, D], f32, kind="ExternalInput")
    rowsq_dram = nc.dram_tensor("rowsq", [P, T], f32, kind="ExternalOutput")
    colsum_dram = nc.dram_tensor("colsum", [1, D], f32, kind="ExternalOutput")

    X = nc.alloc_sbuf_tensor("X", [P, T, D], f32)
    rowsq = nc.alloc_sbuf_tensor("rowsq_sb", [P, T], f32)
    xsq_a = nc.alloc_sbuf_tensor("xsq_a", [P, D], f32)
    xsq_b = nc.alloc_sbuf_tensor("xsq_b", [P, D], f32)
    pairs = [nc.alloc_sbuf_tensor(f"pair{k}", [P, D], bf16) for k in range(4)]
    colsum = nc.alloc_sbuf_tensor("colsum_sb", [1, D], f32)
    ps = nc.alloc_psum_tensor("ps", [1, D], f32)
    onesb = nc.const_aps.tensor(1.0, [P, 1], bf16)

    sA = nc.alloc_semaphore("sA")
    sB = nc.alloc_semaphore("sB")
    sSq = nc.alloc_semaphore("sSq")
    sPr = nc.alloc_semaphore("sPr")
    sMM = nc.alloc_semaphore("sMM")
    sC = nc.alloc_semaphore("sC")
    sOut = nc.alloc_semaphore("sOut")

    x_r = x_dram[:].rearrange("(p t) d -> p t d", p=P)

    nc.sync.dma_start(X[:, 4:6, :], x_r[:, 4:6, :]).then_inc(sB, 16)
    nc.scalar.dma_start(X[:, 0:4, :], x_r[:, 0:4, :]).then_inc(sA, 16)
    nc.scalar.dma_start(X[:, 6:8, :], x_r[:, 6:8, :]).then_inc(sA, 16)

    def act_square(t, col):
        nc.scalar.activation(
            xsq_a[:],
            X[:, t, :],
            mybir.ActivationFunctionType.Square,
            accum_out=rowsq[:, col : col + 1],
        ).then_inc(sSq, 1)

    def dve_square(t, col):
        nc.vector.scalar_tensor_tensor(
            out=xsq_b[:],
            in0=X[:, t, :],
            scalar=1.0,
            in1=X[:, t, :],
            op0=mybir.AluOpType.mult,
            op1=mybir.AluOpType.mult,
            accum_out=rowsq[:, col : col + 1],
        ).then_inc(sSq, 1)

    # DVE
    nc.vector.wait_ge(sB, 16)
    nc.vector.tensor_add(pairs[0][:], X[:, 4, :], X[:, 5, :]).then_inc(sPr, 1)
    dve_square(5, 0)
    nc.vector.wait_ge(sA, 16)
    nc.vector.tensor_add(pairs[1][:], X[:, 0, :], X[:, 1, :]).then_inc(sPr, 1)
    nc.vector.tensor_add(pairs[2][:], X[:, 2, :], X[:, 3, :]).then_inc(sPr, 1)
    dve_square(1, 1)
    nc.vector.wait_ge(sA, 32)
    nc.vector.tensor_add(pairs[3][:], X[:, 6, :], X[:, 7, :]).then_inc(sPr, 1)
    dve_square(3, 2)

    # ACT
    nc.scalar.wait_ge(sB, 16)
    act_square(4, 3)
    nc.scalar.wait_ge(sA, 16)
    act_square(0, 4)
    act_square(2, 5)
    nc.scalar.wait_ge(sA, 32)
    act_square(6, 6)
    act_square(7, 7)
    nc.scalar.wait_ge(sMM, 1)
    nc.scalar.copy(colsum[:], ps[:]).then_inc(sC, 1)
    nc.scalar.wait_ge(sC, 1)
    nc.scalar.dma_start(colsum_dram[:], colsum[:]).then_inc(sOut, 16)

    # PE
    for k in range(4):
        nc.tensor.wait_ge(sPr, k + 1)
        mm = nc.tensor.matmul(ps[:], onesb, pairs[k][:], start=(k == 0), stop=(k == 3))
    mm.then_inc(sMM, 1)

    # SP
    nc.sync.wait_ge(sSq, 8)
    nc.sync.dma_start(rowsq_dram[:], rowsq[:]).then_inc(sOut, 16)
    # Hold the kernel open until both output DMAs complete: without this,
    # the output flight races the NEFF teardown / host readback and loses
    # under heavy co-tenant DMA load (observed: rel err 5.9e+02 once in
    # ~60 runs).  Costs ~1.5us vs the unguarded version.
    nc.sync.wait_ge(sOut, 32)

    nc.compile()
    return nc


def get_nc():
    if not _nc_cache:
        _nc_cache.append(_build_nc())
    return _nc_cache[0]


def combine_partials(rowsq_parts, colsum_parts):
    """rowsq_parts: per-core (P, T) row-squared-norm arrays; colsum_parts:
    per-core (1, D) column sums -> scalar loss.  Row order is irrelevant
    for sum/max, so no reindexing is needed."""
    S = 0.0
    maxsq = -np.inf
    for r in rowsq_parts:
        S += r.sum(dtype=np.float64)
        maxsq = max(maxsq, float(r.max()))
    s = np.zeros(D, dtype=np.float64)
    for cs in colsum_parts:
        s += cs.reshape(-1).astype(np.float64)
    count = N * (N - 1) // 2
    loss = (N * S - s @ s) / (np.sqrt(maxsq) * count)
    return np.float32(loss)


def kernel(x):
    from concourse.bass_utils import run_bass_kernel_spmd

    x = np.ascontiguousarray(np.asarray(x), dtype=np.float32)
    assert x.shape == (N, D), x.shape
    nc = get_nc()
    in_maps = [{"x_blk": x[c * ROWS : (c + 1) * ROWS]} for c in range(NCORES)]
    res = run_bass_kernel_spmd(nc, in_maps, list(range(NCORES)))
    rowsq_parts = [r["rowsq"] for r in res.results]
    colsum_parts = [r["colsum"] for r in res.results]
    return combine_partials(rowsq_parts, colsum_parts)


# revision 6
# speedup vs baseline: 1.0082x; 1.0082x over previous
"""Trainium2 Bass kernel for nn_NormalizedDistanceLoss.

Math: for x in R^{N x D}, with sq_i = ||x_i||^2, the strict-upper-triangle
sum of pairwise squared distances collapses algebraically:

    sum_{i<j} (sq_i + sq_j - 2 x_i.x_j) = N * S - ||s||^2

where S = sum_i sq_i and s = sum_i x_i (column sums).  So the loss

    loss = sum_masked_dist / (sqrt(max_i sq_i) * N(N-1)/2)

needs only one pass over x: per-row squared norms (for S and the max)
and column sums (for s).  Each of the 8 cores reduces its 1024-row block;
the host combines tiny per-core partials (a few KB per core).

Per-core device kernel (block = 1024 x 512 f32), raw bass (no TileContext,
manual semaphores) to minimize fixed framework overhead:
  - SBUF X[128, 8, 512]: partition p holds DRAM rows p*8..p*8+7 (16KB
    contiguous per partition).
  - Input DMA: each HWDGE queue generates descriptors at only ~27ns each,
    so chunk layout balances descriptor counts (128 per chunk): tiles 4-5
    on the SP ring set (4KB descriptors, lands first), then tiles 0-3
    (8KB descriptors) and tiles 6-7 (4KB) both on the Act ring set.  The
    last chunk carries only one pair-add and two squares of tail work.
  - Row squared norms: fused square+row-accumulate; ACT (Square
    activation + accum_out) takes 5 tiles, DVE (scalar_tensor_tensor +
    accum_out) takes 3 plus the pair adds, with the last pair's add
    hoisted ahead of the remaining DVE square so the final matmul ->
    PSUM-copy -> colsum-DMA chain starts as early as possible.
  - Column sums: DVE adds tile pairs into bf16 tiles; the otherwise-idle
    PE contracts the 128 partitions with a ones-vector matmul accumulated
    in one PSUM bank; ACT copies PSUM->SBUF.  bf16 pair rounding perturbs
    the loss ~1e-8 relative.
  - The kernel holds the SP engine until both output DMAs signal
    completion.  (Letting the output flight overlap the fixed ~6us
    semaphore-clear teardown saves ~1.5us but loses a race under heavy
    co-tenant DMA load — observed one corrupted result in ~60 runs.)
"""

import sys

if "/opt/trn_rl_repo" not in sys.path:
    sys.path.insert(0, "/opt/trn_rl_repo")

import numpy as np

from concourse import bacc, mybir

N = 8192
D = 512
NCORES = 8
ROWS = N // NCORES  # 1024 rows per core
P = 128
T = ROWS // P  # 8 row-tiles of 512

_nc_cache = []


def _build_nc():
    f32 = mybir.dt.float32
    bf16 = mybir.dt.bfloat16
    nc = bacc.Bacc(
        "TRN2",
        target_bir_lowering=False,
        debug=False,
        num_devices=NCORES,
    )
    x_dram = nc.dram_tensor("x_blk", [ROWS, D], f32, kind="ExternalInput")
    rowsq_dram = nc.dram_tensor("rowsq", [P, T], f32, kind="ExternalOutput")
    colsum_dram = nc.dram_tensor("colsum", [1, D], f32, kind="ExternalOutput")

    X = nc.alloc_sbuf_tensor("X", [P, T, D], f32)
    rowsq = nc.alloc_sbuf_tensor("rowsq_sb", [P, T], f32)
    xsq_a = nc.alloc_sbuf_tensor("xsq_a", [P, D], f32)
    xsq_b = nc.alloc_sbuf_tensor("xsq_b", [P, D], f32)
    pairs = [nc.alloc_sbuf_tensor(f"pair{k}", [P, D], bf16) for k in range(4)]
    colsum = nc.alloc_sbuf_tensor("colsum_sb", [1, D], f32)
    ps = nc.alloc_psum_tensor("ps", [1, D], f32)
    onesb = nc.const_aps.tensor(1.0, [P, 1], bf16)

    sA = nc.alloc_semaphore("sA")
    sB = nc.alloc_semaphore("sB")
    sSq = nc.alloc_semaphore("sSq")
    sPr = nc.alloc_semaphore("sPr")
    sMM = nc.alloc_semaphore("sMM")
    sC = nc.alloc_semaphore("sC")
    sOut = nc.alloc_semaphore("sOut")

    x_r = x_dram[:].rearrange("(p t) d -> p t d", p=P)

    nc.sync.dma_start(X[:, 4:6, :], x_r[:, 4:6, :]).then_inc(sB, 16)
    nc.scalar.dma_start(X[:, 0:4, :], x_r[:, 0:4, :]).then_inc(sA, 16)
    nc.scalar.dma_start(X[:, 6:8, :], x_r[:, 6:8, :]).then_inc(sA, 16)

    def act_square(t, col):
        nc.scalar.activation(
            xsq_a[:],
            X[:, t, :],
            mybir.ActivationFunctionType.Square,
            accum_out=rowsq[:, col : col + 1],
        ).then_inc(sSq, 1)

    def dve_square(t, col):
        nc.vector.scalar_tensor_tensor(
            out=xsq_b[:],
            in0=X[:, t, :],
            scalar=1.0,
            in1=X[:, t, :],
            op0=mybir.AluOpType.mult,
            op1=mybir.AluOpType.mult,
            accum_out=rowsq[:, col : col + 1],
        ).then_inc(sSq, 1)

    # DVE
    nc.vector.wait_ge(sB, 16)
    nc.vector.tensor_add(pairs[0][:], X[:, 4, :], X[:, 5, :]).then_inc(sPr, 1)
    dve_square(5, 0)
    nc.vector.wait_ge(sA, 16)
    nc.vector.tensor_add(pairs[1][:], X[:, 0, :], X[:, 1, :]).then_inc(sPr, 1)
    nc.vector.tensor_add(pairs[2][:], X[:, 2, :], X[:, 3, :]).then_inc(sPr, 1)
    dve_square(1, 1)
    nc.vector.wait_ge(sA, 32)
    nc.vector.tensor_add(pairs[3][:], X[:, 6, :], X[:, 7, :]).then_inc(sPr, 1)
    dve_square(3, 2)

    # ACT
    nc.scalar.wait_ge(sB, 16)
    act_square(4, 3)
    nc.scalar.wait_ge(sA, 16)
    act_square(0, 4)
    act_square(2, 5)
    nc.scalar.wait_ge(sA, 32)
    act_square(6, 6)
    act_square(7, 7)
    nc.scalar.wait_ge(sMM, 1)
    nc.scalar.copy(colsum[:], ps[:]).then_inc(sC, 1)
    nc.scalar.wait_ge(sC, 1)
    nc.scalar.dma_start(colsum_dram[:], colsum[:]).then_inc(sOut, 16)

    # PE
    for k in range(4):
        nc.tensor.wait_ge(sPr, k + 1)
        mm = nc.tensor.matmul(ps[:], onesb, pairs[k][:], start=(k == 0), stop=(k == 3))
    mm.then_inc(sMM, 1)

    # SP
    nc.sync.wait_ge(sSq, 8)
    nc.sync.dma_start(rowsq_dram[:], rowsq[:]).then_inc(sOut, 16)
    # Hold the kernel open until both output DMAs complete: without this,
    # the output flight races the NEFF teardown / host readback and loses
    # under heavy co-tenant DMA load (observed: rel err 5.9e+02 once in
    # ~60 runs).  Costs ~1.5us vs the unguarded version.
    nc.sync.wait_ge(sOut, 32)

    nc.compile()
    return nc


def get_nc():
    if not _nc_cache:
        _nc_cache.append(_build_nc())
    return _nc_cache[0]


def combine_partials(rowsq_parts, colsum_parts):
    """rowsq_parts: per-core (P, T) row-squared-norm arrays; colsum_parts:
    per-core (1, D) column sums -> scalar loss.  Row order is irrelevant
    for sum/max, so no reindexing is needed."""
    S = 0.0
    maxsq = -np.inf
    for r in rowsq_parts:
        S += r.sum(dtype=np.float64)
        maxsq = max(maxsq, float(r.max()))
    s = np.zeros(D, dtype=np.float64)
    for cs in colsum_parts:
        s += cs.reshape(-1).astype(np.float64)
    count = N * (N - 1) // 2
    loss = (N * S - s @ s) / (np.sqrt(maxsq) * count)
    return np.float32(loss)


def kernel(x):
    from concourse.bass_utils import run_bass_kernel_spmd

    x = np.ascontiguousarray(np.asarray(x), dtype=np.float32)
    assert x.shape == (N, D), x.shape
    nc = get_nc()
    in_maps = [{"x_blk": x[c * ROWS : (c + 1) * ROWS]} for c in range(NCORES)]
    res = run_bass_kernel_spmd(nc, in_maps, list(range(NCORES)))
    rowsq_parts = [r["rowsq"] for r in res.results]
    colsum_parts = [r["colsum"] for r in res.results]
    return combine_partials(rowsq_parts, colsum_parts)
